# revision 1
# baseline (speedup 1.0000x reference)
"""Chamfer loss kernel for Trainium2 (8 NeuronCores, data-parallel over batch).

Problem: x [32, 2048, 3], y [32, 2048, 3] fp32.
  dist[b, m, n] = ||x[b, n] - y[b, m]||^2
  row[b] = mean_n min_m dist ; col[b] = mean_m min_n dist
  out = mean_b max(row, col)

Per core (4 batches): dist[m, n] = yfeat[:, m] . xfeat[:, n], K=5 features
  yfeat = [y0, y1, y2, ||y||^2, 1], xfeat = [-2*x0, -2*x1, -2*x2, 1, ||x||^2]
PE -> PSUM [128 m x 512 n] fp32 tiles; DVE reduces:
  colmin (min over n, free axis) via tensor_reduce per tile,
  rowacc (min over m-chunks, elementwise) via tensor_tensor min into fp16.
Host: rowmin[n] = min_p rowacc[p, n]; means; max; mean over batch.
"""

import os
import sys

import numpy as np

if "/opt/trn_rl_repo" not in sys.path:
    sys.path.insert(0, "/opt/trn_rl_repo")

B, N, M, D = 32, 2048, 2048, 3
N_CORES = 8
BPC = B // N_CORES  # batches per core = 4
MCH = 16  # m-chunks of 128
NCH = 4  # n-chunks of 512

_CACHE = {}
LAST_RESULTS = None


def _build_bass(repeats=1):
    import concourse.bass as bass
    import concourse.tile as tile
    from concourse import mybir

    F32 = mybir.dt.float32
    F16 = mybir.dt.float16
    MIN = mybir.AluOpType.min

    nc = bass.Bass()
    # feats[0] = xfeat [BPC, 5, N], feats[1] = yfeat [BPC, 5, M]
    feats = nc.dram_tensor("feats", [2, BPC, 5, N], F32, kind="ExternalInput")
    # out16[:, : BPC*N]   = rowacc: [p, b, n] -> min over i of dist[128*i+p, n]
    # out16[:, BPC*N :]   = colmin: [p, b*16+i] -> min over n of dist[128*i+p, n]
    out16 = nc.dram_tensor(
        "out16", [128, BPC * N + BPC * MCH], F16, kind="ExternalOutput"
    )

    with tile.TileContext(nc) as tc:
        with (
            tc.tile_pool(name="feat", bufs=1) as featp,
            tc.tile_pool(name="psum", bufs=8, space="PSUM") as psump,
            tc.tile_pool(name="acc", bufs=1) as accp,
            tc.tile_pool(name="colp", bufs=4) as colpp,
        ):
            ft = featp.tile([5, 2 * BPC, N], F32, tag="ft")
            in_dma = nc.sync.dma_start(
                out=ft[:], in_=feats[:].rearrange("t b k n -> k (t b) n")
            )
            xft = ft[:, 0:BPC, :]
            yft = ft[:, BPC : 2 * BPC, :]

            packed = accp.tile([128, BPC * N + BPC * MCH], F16, tag="packed")
            racc = packed[:, 0 : BPC * N].rearrange("p (b n) -> p b n", b=BPC)
            colsb = packed[:, BPC * N :]

            last_mm = None
            for _r in range(repeats):
              for b in range(BPC):
                for i in range(MCH):
                    colpart = colpp.tile([128, NCH], F32, tag="colpart")
                    for j in range(NCH):
                        ps = psump.tile([128, 512], F32, tag="ps")
                        last_mm = nc.tensor.matmul(
                            ps[:],
                            yft[:, b, 128 * i : 128 * (i + 1)],
                            xft[:, b, 512 * j : 512 * (j + 1)],
                            start=True,
                            stop=True,
                        )
                        nc.vector.tensor_reduce(
                            out=colpart[:, j : j + 1],
                            in_=ps[:],
                            axis=mybir.AxisListType.X,
                            op=MIN,
                        )
                        if i == 0:
                            nc.vector.tensor_copy(
                                racc[:, b, 512 * j : 512 * (j + 1)], ps[:]
                            )
                        else:
                            nc.vector.tensor_tensor(
                                racc[:, b, 512 * j : 512 * (j + 1)],
                                ps[:],
                                racc[:, b, 512 * j : 512 * (j + 1)],
                                MIN,
                            )
                    last_dve = nc.vector.tensor_reduce(
                        out=colsb[:, b * MCH + i : b * MCH + i + 1],
                        in_=colpart[:],
                        axis=mybir.AxisListType.X,
                        op=MIN,
                    )
            # Pre-observe PE and the input-DMA lane on the SP engine so the
            # Tile end-of-kernel Drain needs only 2 sem waits (DVE + out-DMA);
            # walrus rejects instructions with >2 sync waits.
            from concourse.tile_rust import add_dep_helper

            nop1 = nc.sync.nop(nofuse=True)
            add_dep_helper(nop1.ins, last_mm.ins, sync=True, reason="observe PE")
            nop2 = nc.sync.nop(nofuse=True)
            add_dep_helper(nop2.ins, in_dma.ins, sync=True, reason="observe in-dma")
            nc.sync.dma_start(out=out16[:], in_=packed[:])

    # The Tile end-of-kernel SP Drain waits on every outstanding proc, but
    # walrus only allows 1 sync wait on a Drain. Engine waits (PE/DVE) are
    # redundant with the all-engine barrier that follows the drain (each
    # engine's barrier inc is program-ordered after its last op) — keep only
    # DMA-lane waits, which the barrier does not cover.
    for fn in nc.m.functions:
        for bb in fn.blocks:
            for ins in bb.instructions:
                if ins.__class__.__name__ == "InstDrain" and ins.sync_info is not None:
                    w = ins.sync_info.on_wait
                    if len(w) > 1:
                        keep = [x for x in w if x.ant_name.startswith("DMA")]
                        assert len(keep) <= 1, [x.ant_name for x in w]
                        ins.sync_info.on_wait = keep

    return nc


def _prep_core_inputs(x, y, c):
    xb = x[BPC * c : BPC * (c + 1)]  # [4, 2048, 3]
    yb = y[BPC * c : BPC * (c + 1)]
    ones = np.ones((BPC, N), np.float32)
    x2 = np.sum(xb.astype(np.float32) ** 2, axis=-1)  # [4, N]
    y2 = np.sum(yb.astype(np.float32) ** 2, axis=-1)  # [4, M]
    xfeat = np.stack(
        [-2.0 * xb[..., 0], -2.0 * xb[..., 1], -2.0 * xb[..., 2], ones, x2], axis=1
    ).astype(np.float32)  # [4, 5, N]
    yfeat = np.stack(
        [yb[..., 0], yb[..., 1], yb[..., 2], y2, ones], axis=1
    ).astype(np.float32)  # [4, 5, M]
    return np.ascontiguousarray(np.stack([xfeat, yfeat], axis=0))  # [2, 4, 5, N]


def kernel(x, y):
    global LAST_RESULTS
    from concourse.bass_utils import run_bass_kernel_spmd

    x = np.asarray(x, dtype=np.float32)
    y = np.asarray(y, dtype=np.float32)
    assert x.shape == (B, N, D) and y.shape == (B, M, D)

    if "nc" not in _CACHE:
        _CACHE["nc"] = _build_bass()
    nc = _CACHE["nc"]

    in_maps = []
    for c in range(N_CORES):
        in_maps.append({"feats": _prep_core_inputs(x, y, c)})

    res = run_bass_kernel_spmd(nc, in_maps, core_ids=list(range(N_CORES)))
    LAST_RESULTS = res

    cham = np.zeros((B,), np.float64)
    for c in range(N_CORES):
        out = res.results[c]["out16"]  # [128, BPC*N + BPC*MCH] fp16
        rowacc = out[:, : BPC * N].reshape(128, BPC, N)
        colmin = out[:, BPC * N :].reshape(128, BPC, MCH)
        rowmin = rowacc.min(axis=0).astype(np.float64)  # [4, N]
        row = rowmin.mean(axis=1)  # [4]
        for b in range(BPC):
            col = colmin[:, b, :].astype(np.float64).mean()
            cham[BPC * c + b] = max(row[b], col)
    return np.float32(cham.mean())



# revision 24
# speedup vs baseline: 67.7927x; 67.7927x over previous
"""Chamfer loss kernel for Trainium2 (8 NeuronCores, data-parallel over batch).

Problem: x [32, 2048, 3], y [32, 2048, 3] fp32.
  dist[b, m, n] = ||x[b, n] - y[b, m]||^2
  row[b] = mean_n min_m dist ; col[b] = mean_m min_n dist
  out = mean_b max(row, col)

Per core (4 batches): dist[m, n] = yfeat[:, m] . xfeat[:, n], K=5 features
  yfeat = [y0, y1, y2, ||y||^2, 1], xfeat = [-2*x0, -2*x1, -2*x2, 1, ||x||^2]
PE -> PSUM [128 m x 512 n] fp32 tiles.

v2 engine split (vs v1 which did everything on DVE from PSUM at 1x):
  - Most (b, i) groups: ACT evacuates the 4 PSUM tiles to fp16 SBUF; DVE
    then runs at 2x on fp16: col pre-combine (3 TT-min + 1 reduce per
    group) and row accumulation (TT-min into racc); a subset of groups'
    row TTs go to GPSIMD to offload DVE.
  - A few groups stay on the v1 PSUM-direct path (T1) to soak spare DVE.
  - i == 0 evacuates straight into racc (row init for free).
Host: rowmin[n] = min_p racc[p, n]; means; max; mean over batch.
"""

import os
import sys

import numpy as np

if "/opt/trn_rl_repo" not in sys.path:
    sys.path.insert(0, "/opt/trn_rl_repo")

B, N, M, D = 32, 2048, 2048, 3
N_CORES = 8
BPC = B // N_CORES  # batches per core = 4
MCH = 16  # m-chunks of 128
NCH = 4  # n-chunks of 512

# --- engine assignment knobs -------------------------------------------------
# groups are (b, i), enumerated g = b*MCH + i, i in [0, 16)
# To keep every instruction at <=2 semaphore waits, each group's SBUF ops all
# run on ONE engine (DVE or gpsimd); only the free-axis reduce is DVE-only.
N_T1 = 0  # groups on the PSUM-direct DVE path (no evac)
N_GP_ROW = 0  # gpsimd TT is rejected by this walrus build; keep 0


def _paths():
    """Return (t1set, gprowset) of group indices."""
    ngroups = BPC * MCH
    # T1 groups: spread out, never i == 0
    t1 = []
    k = 5
    while len(t1) < N_T1:
        if k % MCH != 0 and k not in t1:
            t1.append(k)
        k = (k + 7) % ngroups
    t1 = set(t1)
    # gpsimd row groups: evenly spread over remaining i>0 groups. Exclude
    # g-1 neighbors of T1 groups so a T1 group's PSUM-direct row-TT only ever
    # chains racc from a DVE-written (or ACT-written) slice — keeps its
    # cross-engine wait count at 1 (just PE).
    excl = t1 | {x - 1 for x in t1}
    cands = [g for g in range(ngroups) if g % MCH != 0 and g not in excl]
    n = min(N_GP_ROW, len(cands))
    gp = set(cands[(k * len(cands)) // max(n, 1)] for k in range(n))
    return t1, gp


_CACHE = {}
LAST_RESULTS = None


def _build_bass(repeats=1):
    import concourse.bass as bass
    import concourse.tile as tile
    from concourse import mybir

    F32 = mybir.dt.float32
    F16 = mybir.dt.float16
    MIN = mybir.AluOpType.min

    t1set, gpset = _paths()

    nc = bass.Bass()
    # feats[0] = xfeat [BPC, 5, N], feats[1] = yfeat [BPC, 5, M]
    feats = nc.dram_tensor("feats", [2, BPC, 5, N], F32, kind="ExternalInput")
    # out16[:, : BPC*N]   = racc: [p, b, n] -> min over i of dist[128*i+p, n]
    # out16[:, BPC*N :]   = colmin: [p, b*16+i] -> min over n of dist[128*i+p, n]
    out16 = nc.dram_tensor(
        "out16", [128, BPC * N + BPC * MCH], F16, kind="ExternalOutput"
    )

    from concourse.tile_rust import add_dep_helper

    EV_BUFS = 4
    CC_BUFS = 4

    with tile.TileContext(nc) as tc:
        with (
            tc.tile_pool(name="feat", bufs=1) as featp,
            tc.tile_pool(name="psum", bufs=8, space="PSUM") as psump,
            tc.tile_pool(name="acc", bufs=1) as accp,
            tc.tile_pool(name="colcd", bufs=CC_BUFS) as colcp_dve,
            tc.tile_pool(name="colp", bufs=4) as colpp,
        ):
            ft = featp.tile([5, 2 * BPC, N], F32, tag="ft")
            in_dma = nc.sync.dma_start(
                out=ft[:], in_=feats[:].rearrange("t b k n -> k (t b) n")
            )
            xft = ft[:, 0:BPC, :]
            yft = ft[:, BPC : 2 * BPC, :]

            packed = accp.tile([128, BPC * N + BPC * MCH], F16, tag="packed")
            racc = packed[:, 0 : BPC * N].rearrange("p (b n) -> p b n", b=BPC)
            colf = packed[:, BPC * N :]
            cmin = accp.tile([128, BPC * MCH], F32, tag="cmin")
            # deterministic round-robin evac slots / gp col-combine slots
            # (manual reuse so the WAR target is known and can be
            # pre-observed on the writing engine with a 1-wait nop)
            evbuf = accp.tile([128, EV_BUFS * N], F16, tag="evbuf")
            ccgbuf = accp.tile([128, CC_BUFS * 512], F16, tag="ccgbuf")
            scrap = accp.tile([128, NCH], F16, tag="scrap")
            scrapg = accp.tile([128, 1], F16, tag="scrapg")

            ev_last = [None] * EV_BUFS  # last consumer op per ev slot
            ccg_last = [None] * CC_BUFS  # reduce op per gp cc slot
            racc_last = [None] * BPC  # last row-TT of (b, 15)
            ev_ctr = 0
            ccg_ctr = 0
            last_mm = None
            last_pool_op = None

            for _r in range(repeats):
              for b in range(BPC):
                for i in range(MCH):
                    g = b * MCH + i
                    t1 = g in t1set
                    gp_row = g in gpset
                    # evac target: racc slices for i == 0, else an ev slot
                    war_obs = None
                    if t1:
                        ev = None
                    elif i == 0:
                        ev = racc[:, b, :]
                        war_obs = racc_last[b]
                        slot = None
                    else:
                        slot = ev_ctr % EV_BUFS
                        ev_ctr += 1
                        ev = evbuf[:, slot * N : (slot + 1) * N]
                        war_obs = ev_last[slot]
                    # pre-observe the evac target's previous consumer on ACT
                    # (1 wait) so the evacuations themselves carry only the
                    # PE wait. A real ACT read of a sliver overlapping every
                    # 512-slice makes Tile order all 4 evac writes after it.
                    if war_obs is not None and ev is not None:
                        obs = nc.scalar.copy(scrap[:], ev[:, 0 : N : 512])
                        add_dep_helper(
                            obs.ins, war_obs.ins, sync=True,
                            reason="observe evac WAR",
                        )

                    if t1:
                        colpart = colpp.tile([128, NCH], F32, tag="colpart")
                    for j in range(NCH):
                        ps = psump.tile([128, 512], F32, tag="ps")
                        last_mm = nc.tensor.matmul(
                            ps[:],
                            yft[:, b, 128 * i : 128 * (i + 1)],
                            xft[:, b, 512 * j : 512 * (j + 1)],
                            start=True,
                            stop=True,
                        )
                        if t1:
                            # v1 path: reduce + row-TT straight from PSUM
                            red = nc.vector.tensor_reduce(
                                out=colpart[:, j : j + 1],
                                in_=ps[:],
                                axis=mybir.AxisListType.X,
                                op=MIN,
                            )
                            tt = nc.vector.tensor_tensor(
                                racc[:, b, 512 * j : 512 * (j + 1)],
                                ps[:],
                                racc[:, b, 512 * j : 512 * (j + 1)],
                                MIN,
                            )
                            # keep reduce -> TT order so the TT's PE wait is
                            # already observed by the reduce
                            add_dep_helper(
                                tt.ins, red.ins, sync=True, reason="order"
                            )
                            if i == MCH - 1 and j == NCH - 1:
                                racc_last[b] = tt
                        else:
                            nc.scalar.copy(
                                ev[:, 512 * j : 512 * (j + 1)], ps[:]
                            )
                    if t1:
                        nc.vector.tensor_reduce(
                            out=cmin[:, g : g + 1],
                            in_=colpart[:],
                            axis=mybir.AxisListType.X,
                            op=MIN,
                        )
                        continue

                    # col pre-combine (fp16) + one DVE reduce
                    eng = nc.gpsimd if gp_row else nc.vector
                    if gp_row:
                        slot2 = ccg_ctr % CC_BUFS
                        ccg_ctr += 1
                        cc = ccgbuf[:, slot2 * 512 : (slot2 + 1) * 512]
                        if ccg_last[slot2] is not None:
                            obsg = nc.gpsimd.tensor_copy(scrapg[:], cc[:, 0:1])
                            add_dep_helper(
                                obsg.ins, ccg_last[slot2].ins, sync=True,
                                reason="observe gp cc WAR",
                            )
                        eng.tensor_copy(cc, ev[:, 0:512])
                    else:
                        cct = colcp_dve.tile([128, 512], F16, tag="ccd", name="ccd")
                        cc = cct[:]
                        eng.tensor_copy(cc, ev[:, 0:512])
                    for j in range(1, NCH):
                        eng.tensor_tensor(
                            cc, ev[:, 512 * j : 512 * (j + 1)], cc, MIN
                        )
                    red = nc.vector.tensor_reduce(
                        out=cmin[:, g : g + 1],
                        in_=cc,
                        axis=mybir.AxisListType.X,
                        op=MIN,
                    )
                    if gp_row:
                        ccg_last[slot2] = red

                    # row accumulation (skip for i == 0: evac'd into racc)
                    if i > 0:
                        tt = None
                        for j in range(NCH):
                            sl = slice(512 * j, 512 * (j + 1))
                            tt = eng.tensor_tensor(
                                racc[:, b, sl], ev[:, sl], racc[:, b, sl], MIN
                            )
                        ev_last[slot] = tt
                        if gp_row:
                            last_pool_op = tt
                        if i == MCH - 1:
                            racc_last[b] = tt
                    else:
                        # i == 0: ev slices are consumed by the col ops; the
                        # last consumer is the copy/TTs (eng). Use the last
                        # col TT as racc[b]'s guard only if needed later —
                        # i == 0 writes are superseded by later row TTs.
                        pass
              # cast colmins fp32 -> fp16 into the packed output; order it
              # after the last Pool op so the output DMA's Pool dependency
              # is covered transitively through its DVE wait
              last_dve = nc.vector.tensor_copy(colf[:], cmin[:])
              if last_pool_op is not None:
                  add_dep_helper(
                      last_dve.ins, last_pool_op.ins, sync=True,
                      reason="observe Pool before out-dma",
                  )

            # Pre-observe engines/DMA lanes on SP so the output DMA and the
            # Tile end-of-kernel Drain each need <=1 sem wait.
            nop1 = nc.sync.nop(nofuse=True)
            add_dep_helper(nop1.ins, last_mm.ins, sync=True, reason="observe PE")
            nop2 = nc.sync.nop(nofuse=True)
            add_dep_helper(nop2.ins, in_dma.ins, sync=True, reason="observe in-dma")
            if last_pool_op is not None:
                nop3 = nc.sync.nop(nofuse=True)
                add_dep_helper(
                    nop3.ins, last_pool_op.ins, sync=True, reason="observe Pool"
                )
            nc.sync.dma_start(out=out16[:], in_=packed[:])

    # Walrus encodes at most 2 sem-wait commands per instruction (1 on a
    # Drain). Tile's redundant-wait eliminator (optimize_sems) is disabled,
    # so instructions can carry a redundant SAME-ENGINE wait: the compute
    # engines (ACT/DVE/Pool) execute their queues serially-complete, so a
    # wait on the instruction's own engine semaphore is vacuous — strip it
    # when over budget. Never strips DMA-queue waits.
    # The end-of-kernel SP Drain additionally drops engine waits, which are
    # redundant with the all-engine barrier that follows the drain (each
    # engine's barrier inc is program-ordered after its last op); only
    # DMA-lane waits are kept there.
    own_prefix = {
        mybir.EngineType.Activation: "Activation_",
        mybir.EngineType.DVE: "DVE_",
        mybir.EngineType.Pool: "Pool_",
    }
    for fn in nc.m.functions:
        for bb in fn.blocks:
            # sems already waited on by earlier SP-queue instructions in this
            # block. The SP nops wait on the FINAL op of their engine, so a
            # later SP wait on the same semaphore is covered.
            sp_seen = set()
            for ins in bb.instructions:
                si = getattr(ins, "sync_info", None)
                if si is None:
                    continue
                engname = str(getattr(ins, "engine", "")).split(".")[-1]
                if ins.__class__.__name__ == "InstDrain":
                    w = si.on_wait
                    if len(w) > 1:
                        keep = [x for x in w if x.ant_name.startswith("DMA")]
                        assert len(keep) <= 1, [x.ant_name for x in w]
                        si.on_wait = keep
                    continue
                w = si.on_wait
                if engname == "SP" and ins.__class__.__name__ == "InstNoOp":
                    sp_seen.update(x.ant_name for x in w)
                if len(w) > 1:
                    pfx = own_prefix.get(ins.engine)
                    if pfx is not None:
                        w = [x for x in w if not x.ant_name.startswith(pfx)]
                    if len(w) > 1 and ins.__class__.__name__ == "InstDMACopy":
                        w = [x for x in w if x.ant_name not in sp_seen]
                    si.on_wait = w
                    assert len(w) <= 1 or ins.__class__.__name__ == "InstDMACopy", (
                        ins.__class__.__name__,
                        [x.ant_name for x in si.on_wait],
                    )

    return nc


def _prep_core_inputs(x, y, c):
    xb = x[BPC * c : BPC * (c + 1)]  # [4, 2048, 3]
    yb = y[BPC * c : BPC * (c + 1)]
    ones = np.ones((BPC, N), np.float32)
    x2 = np.sum(xb.astype(np.float32) ** 2, axis=-1)  # [4, N]
    y2 = np.sum(yb.astype(np.float32) ** 2, axis=-1)  # [4, M]
    xfeat = np.stack(
        [-2.0 * xb[..., 0], -2.0 * xb[..., 1], -2.0 * xb[..., 2], ones, x2], axis=1
    ).astype(np.float32)  # [4, 5, N]
    yfeat = np.stack(
        [yb[..., 0], yb[..., 1], yb[..., 2], y2, ones], axis=1
    ).astype(np.float32)  # [4, 5, M]
    return np.ascontiguousarray(np.stack([xfeat, yfeat], axis=0))  # [2, 4, 5, N]


def kernel(x, y):
    global LAST_RESULTS
    from concourse.bass_utils import run_bass_kernel_spmd

    x = np.asarray(x, dtype=np.float32)
    y = np.asarray(y, dtype=np.float32)
    assert x.shape == (B, N, D) and y.shape == (B, M, D)

    if "nc" not in _CACHE:
        _CACHE["nc"] = _build_bass()
    nc = _CACHE["nc"]

    in_maps = []
    for c in range(N_CORES):
        in_maps.append({"feats": _prep_core_inputs(x, y, c)})

    res = run_bass_kernel_spmd(nc, in_maps, core_ids=list(range(N_CORES)))
    LAST_RESULTS = res

    cham = np.zeros((B,), np.float64)
    for c in range(N_CORES):
        out = res.results[c]["out16"]  # [128, BPC*N + BPC*MCH] fp16
        rowacc = out[:, : BPC * N].reshape(128, BPC, N)
        colmin = out[:, BPC * N :].reshape(128, BPC, MCH)
        rowmin = rowacc.min(axis=0).astype(np.float64)  # [4, N]
        row = rowmin.mean(axis=1)  # [4]
        for b in range(BPC):
            col = colmin[:, b, :].astype(np.float64).mean()
            cham[BPC * c + b] = max(row[b], col)
    return np.float32(cham.mean())


# revision 35
# speedup vs baseline: 95.2918x; 1.4056x over previous
"""Chamfer loss kernel for Trainium2 (8 NeuronCores, data-parallel over batch).

Problem: x [32, 2048, 3], y [32, 2048, 3] fp32.
  dist[b, m, n] = ||x[b, n] - y[b, m]||^2
  row[b] = mean_n min_m dist ; col[b] = mean_m min_n dist
  out = mean_b max(row, col)

Per core (4 batches): dist[m, n] = yfeat[:, m] . xfeat[:, n], K=5 features
  yfeat = [y0, y1, y2, ||y||^2, 1], xfeat = [-2*x0, -2*x1, -2*x2, 1, ||x||^2]
PE -> PSUM [128 m x 512 n] fp32 tiles.

v2 engine split (vs v1 which did everything on DVE from PSUM at 1x):
  - Most (b, i) groups: ACT evacuates the 4 PSUM tiles to fp16 SBUF; DVE
    then runs at 2x on fp16: col pre-combine (3 TT-min + 1 reduce per
    group) and row accumulation (TT-min into racc); a subset of groups'
    row TTs go to GPSIMD to offload DVE.
  - A few groups stay on the v1 PSUM-direct path (T1) to soak spare DVE.
  - i == 0 evacuates straight into racc (row init for free).
Host: rowmin[n] = min_p racc[p, n]; means; max; mean over batch.
"""

import os
import sys

import numpy as np

if "/opt/trn_rl_repo" not in sys.path:
    sys.path.insert(0, "/opt/trn_rl_repo")

B, N, M, D = 32, 2048, 2048, 3
N_CORES = 8
BPC = B // N_CORES  # batches per core = 4
MCH = 16  # m-chunks of 128
NCH = 4  # n-chunks of 512

# --- engine assignment knobs -------------------------------------------------
# groups are (b, i), enumerated g = b*MCH + i, i in [0, 16)
# To keep every instruction at <=2 semaphore waits, each group's SBUF ops all
# run on ONE engine (DVE or gpsimd); only the free-axis reduce is DVE-only.
N_T1 = 10  # groups on the PSUM-direct DVE path (no evac)
N_GP_ROW = 0  # gpsimd TT is rejected by this walrus build; keep 0


def _paths():
    """Return (t1set, gprowset) of group indices."""
    ngroups = BPC * MCH
    # T1 groups: spread out, never i == 0
    t1 = []
    k = 5
    while len(t1) < N_T1:
        if k % MCH != 0 and k not in t1:
            t1.append(k)
        k = (k + 7) % ngroups
    t1 = set(t1)
    return t1, set()


_CACHE = {}
LAST_RESULTS = None


def _build_bass(repeats=1):
    import concourse.bass as bass
    import concourse.tile as tile
    from concourse import mybir

    F32 = mybir.dt.float32
    F16 = mybir.dt.float16
    MIN = mybir.AluOpType.min

    t1set, gpset = _paths()

    nc = bass.Bass()
    # feats[0] = xfeat [BPC, 5, N], feats[1] = yfeat [BPC, 5, M]
    feats = nc.dram_tensor("feats", [2, BPC, 5, N], F32, kind="ExternalInput")
    # out16[:, : BPC*N]   = racc: [p, b, n] -> min over i of dist[128*i+p, n]
    # out16[:, BPC*N :]   = colmin: [p, b*16+i] -> min over n of dist[128*i+p, n]
    out16 = nc.dram_tensor(
        "out16", [128, BPC * N + BPC * MCH], F16, kind="ExternalOutput"
    )

    from concourse.tile_rust import add_dep_helper

    EV_BUFS = 12
    SCRAP_EVERY = 12
    CC_BUFS = 4

    with tile.TileContext(nc) as tc:
        with (
            tc.tile_pool(name="feat", bufs=1) as featp,
            tc.tile_pool(name="psum", bufs=2, space="PSUM") as psump,
            tc.tile_pool(name="acc", bufs=1) as accp,
            tc.tile_pool(name="colcd", bufs=CC_BUFS) as colcp_dve,
        ):
            ft = featp.tile([5, 2 * BPC, N], F32, tag="ft")
            in_dma = nc.sync.dma_start(
                out=ft[:], in_=feats[:].rearrange("t b k n -> k (t b) n")
            )
            xft = ft[:, 0:BPC, :]
            yft = ft[:, BPC : 2 * BPC, :]

            # One big SBUF tile holds racc | colf | evac slots so a single
            # strided ACT sliver-read overlaps every future ACT write target.
            OUT_W = BPC * N + BPC * MCH
            packed = accp.tile([128, OUT_W + EV_BUFS * N], F16, tag="packed")
            racc = packed[:, 0 : BPC * N].rearrange("p (b n) -> p b n", b=BPC)
            colf = packed[:, BPC * N : OUT_W]
            evbuf = packed[:, OUT_W:]
            cmin = accp.tile([128, BPC * MCH], F32, tag="cmin")
            NSLIV = (OUT_W + EV_BUFS * N) // 512
            scrapt = accp.tile([128, NSLIV], F16, tag="scrapt")

            racc_last = [None] * BPC  # last row-TT of (b, 15)
            ev_ctr = 0
            grp_ctr = 0
            last_mm = None
            last_dve = None

            for _r in range(repeats):
              for b in range(BPC):
                for i in range(MCH):
                    g = b * MCH + i
                    t1 = g in t1set
                    # Periodic ACT observation of the DVE frontier: one
                    # sliver-read (overlapping every ACT evac target) every
                    # SCRAP_EVERY groups. Its DVE wait advances the ACT
                    # queue's vector clock so the following evacuations'
                    # WAR dependencies (vs DVE consumers of reused slots /
                    # racc) are already observed -> each evac carries only
                    # its PE wait.
                    if grp_ctr % SCRAP_EVERY == 0 and last_dve is not None:
                        obs = nc.scalar.copy(
                            scrapt[:], packed[:, 0 : NSLIV * 512 : 512]
                        )
                        add_dep_helper(
                            obs.ins, last_dve.ins, sync=True,
                            reason="observe DVE frontier",
                        )
                    grp_ctr += 1
                    # evac target: racc slices for i == 0, else an ev slot
                    if t1:
                        ev = None
                    elif i == 0:
                        ev = racc[:, b, :]
                    else:
                        slot = ev_ctr % EV_BUFS
                        ev_ctr += 1
                        ev = evbuf[:, slot * N : (slot + 1) * N]

                    # one wide 4-bank PSUM tile per group, 4 matmuls
                    psw = psump.tile([128, N], F32, tag="psw", name="psw")
                    for j in range(NCH):
                        last_mm = nc.tensor.matmul(
                            psw[:, 512 * j : 512 * (j + 1)],
                            yft[:, b, 128 * i : 128 * (i + 1)],
                            xft[:, b, 512 * j : 512 * (j + 1)],
                            start=True,
                            stop=True,
                        )

                    if t1:
                        # PSUM-direct path: wide reduce (exact colmin for
                        # this m-chunk) + wide row-TT
                        red = nc.vector.tensor_reduce(
                            out=cmin[:, g : g + 1],
                            in_=psw[:],
                            axis=mybir.AxisListType.X,
                            op=MIN,
                        )
                        tt = nc.vector.tensor_tensor(
                            racc[:, b, :], psw[:], racc[:, b, :], MIN
                        )
                        # reduce -> TT order: the TT's PE wait is then
                        # already observed on the DVE queue
                        add_dep_helper(tt.ins, red.ins, sync=True, reason="order")
                        last_dve = tt
                        if i == MCH - 1:
                            racc_last[b] = tt
                        continue

                    # one wide ACT evacuation PSUM -> fp16 SBUF
                    nc.scalar.copy(ev[:], psw[:])

                    # col min-tree on DVE fp16 (2x) + one short reduce
                    cct = colcp_dve.tile([128, N // 2], F16, tag="ccd", name="ccd")
                    cc = cct[:]
                    nc.vector.tensor_tensor(
                        cc, ev[:, 0 : N // 2], ev[:, N // 2 : N], MIN
                    )
                    nc.vector.tensor_tensor(
                        cc[:, 0:512], cc[:, 0:512], cc[:, 512:1024], MIN
                    )
                    nc.vector.tensor_tensor(
                        cc[:, 0:256], cc[:, 0:256], cc[:, 256:512], MIN
                    )
                    last_dve = nc.vector.tensor_reduce(
                        out=cmin[:, g : g + 1],
                        in_=cc[:, 0:256],
                        axis=mybir.AxisListType.X,
                        op=MIN,
                    )

                    # row accumulation (skip for i == 0: evac'd into racc)
                    if i > 0:
                        tt = nc.vector.tensor_tensor(
                            racc[:, b, :], ev[:], racc[:, b, :], MIN
                        )
                        last_dve = tt
                        if i == MCH - 1:
                            racc_last[b] = tt
              # cast colmins fp32 -> fp16 into the packed output
              nc.vector.tensor_copy(colf[:], cmin[:])

            # Pre-observe engines/DMA lanes on SP so the output DMA and the
            # Tile end-of-kernel Drain each need <=1 sem wait.
            nop1 = nc.sync.nop(nofuse=True)
            add_dep_helper(nop1.ins, last_mm.ins, sync=True, reason="observe PE")
            nop2 = nc.sync.nop(nofuse=True)
            add_dep_helper(nop2.ins, in_dma.ins, sync=True, reason="observe in-dma")
            nc.sync.dma_start(out=out16[:], in_=packed[:, 0:OUT_W])

    # Walrus encodes at most 2 sem-wait commands per instruction (1 on a
    # Drain). Tile's redundant-wait eliminator (optimize_sems) is disabled,
    # so instructions can carry a redundant SAME-ENGINE wait: the compute
    # engines (ACT/DVE/Pool) execute their queues serially-complete, so a
    # wait on the instruction's own engine semaphore is vacuous — strip it
    # when over budget. Never strips DMA-queue waits.
    # The end-of-kernel SP Drain additionally drops engine waits, which are
    # redundant with the all-engine barrier that follows the drain (each
    # engine's barrier inc is program-ordered after its last op); only
    # DMA-lane waits are kept there.
    own_prefix = {
        mybir.EngineType.Activation: "Activation_",
        mybir.EngineType.DVE: "DVE_",
        mybir.EngineType.Pool: "Pool_",
        # PE matmul completions are pc-monotone (no reordering of ends), so
        # a matmul's wait on the PE semaphore is equally vacuous.
        mybir.EngineType.PE: "PE_",
    }
    for fn in nc.m.functions:
        for bb in fn.blocks:
            # sems already waited on by earlier SP-queue instructions in this
            # block. The SP nops wait on the FINAL op of their engine, so a
            # later SP wait on the same semaphore is covered.
            sp_seen = set()
            for ins in bb.instructions:
                si = getattr(ins, "sync_info", None)
                if si is None:
                    continue
                engname = str(getattr(ins, "engine", "")).split(".")[-1]
                if ins.__class__.__name__ == "InstDrain":
                    w = si.on_wait
                    if len(w) > 1:
                        keep = [x for x in w if x.ant_name.startswith("DMA")]
                        assert len(keep) <= 1, [x.ant_name for x in w]
                        si.on_wait = keep
                    continue
                w = si.on_wait
                if engname == "SP" and ins.__class__.__name__ == "InstNoOp":
                    sp_seen.update(x.ant_name for x in w)
                if len(w) > 1:
                    pfx = own_prefix.get(ins.engine)
                    if pfx is not None:
                        w = [x for x in w if not x.ant_name.startswith(pfx)]
                    if len(w) > 1 and ins.__class__.__name__ == "InstDMACopy":
                        w = [x for x in w if x.ant_name not in sp_seen]
                    si.on_wait = w
                    # Activation allows 2 wait slots; TT/reduce/copy/matmul 1
                    limit = 2 if ins.__class__.__name__ in ("InstActivation", "InstDMACopy") else 1
                    assert len(w) <= limit, (
                        ins.__class__.__name__,
                        [x.ant_name for x in si.on_wait],
                    )

    return nc


def _prep_core_inputs(x, y, c):
    xb = x[BPC * c : BPC * (c + 1)]  # [4, 2048, 3]
    yb = y[BPC * c : BPC * (c + 1)]
    ones = np.ones((BPC, N), np.float32)
    x2 = np.sum(xb.astype(np.float32) ** 2, axis=-1)  # [4, N]
    y2 = np.sum(yb.astype(np.float32) ** 2, axis=-1)  # [4, M]
    xfeat = np.stack(
        [-2.0 * xb[..., 0], -2.0 * xb[..., 1], -2.0 * xb[..., 2], ones, x2], axis=1
    ).astype(np.float32)  # [4, 5, N]
    yfeat = np.stack(
        [yb[..., 0], yb[..., 1], yb[..., 2], y2, ones], axis=1
    ).astype(np.float32)  # [4, 5, M]
    return np.ascontiguousarray(np.stack([xfeat, yfeat], axis=0))  # [2, 4, 5, N]


def kernel(x, y):
    global LAST_RESULTS
    from concourse.bass_utils import run_bass_kernel_spmd

    x = np.asarray(x, dtype=np.float32)
    y = np.asarray(y, dtype=np.float32)
    assert x.shape == (B, N, D) and y.shape == (B, M, D)

    if "nc" not in _CACHE:
        _CACHE["nc"] = _build_bass()
    nc = _CACHE["nc"]

    in_maps = []
    for c in range(N_CORES):
        in_maps.append({"feats": _prep_core_inputs(x, y, c)})

    res = run_bass_kernel_spmd(nc, in_maps, core_ids=list(range(N_CORES)))
    LAST_RESULTS = res

    cham = np.zeros((B,), np.float64)
    for c in range(N_CORES):
        out = res.results[c]["out16"]  # [128, BPC*N + BPC*MCH] fp16
        rowacc = out[:, : BPC * N].reshape(128, BPC, N)
        colmin = out[:, BPC * N :].reshape(128, BPC, MCH)
        rowmin = rowacc.min(axis=0).astype(np.float64)  # [4, N]
        row = rowmin.mean(axis=1)  # [4]
        for b in range(BPC):
            col = colmin[:, b, :].astype(np.float64).mean()
            cham[BPC * c + b] = max(row[b], col)
    return np.float32(cham.mean())
